# revision 23
# baseline (speedup 1.0000x reference)
"""Trainium2 Bass kernel for nn_CombinedLoss (Poisson + 3-way pairwise CLIP loss).

Strategy (8 NeuronCores, SPMD, no collectives), v3 "flipped orientation":
  - Row-shard the batch: core c owns rows [c*512, (c+1)*512) of every tensor.
  - For each pair (a,b) in {(1,2),(1,3),(2,3)} each core computes the FULL
    column strip S_ab^T[n, m] = <b_n_raw, zhat_a_m> for all 4096 n and its own
    512 m, using fp8(e4m3) DoubleRow matmuls (2x PE throughput, fp32 PSUM):
      lhsT (stationary) = raw fp8 b^T tiles, DMA'd directly from a
             host-side transposed+fp8-cast+pre-tiled copy of the full feature
             (no on-device normalization / transpose / scratch roundtrip);
      rhs  (moving)     = own rows of a, normalized (x16) to fp8 and
             PE-transposed on-chip.
  - The missing 1/||b_n|| normalization is per-PSUM-PARTITION in this
    orientation, so it folds into the Exp's per-partition scale:
    exp(S_raw[n,m] / (16 * T * ||b_n||)).  ||b_n||^2 for all n comes from
    fp8 Gram-diagonal matmuls on the already-loaded b^T tiles (diag extracted
    with a DVE multiply-by-identity + free-dim accumulate).
  - Column sums over own m (partial, host-combined) come free via the Exp's
    accum_out.  Row sums over all n are ones-vector DoubleRow matmuls
    accumulated per (pair, chunk) in a rotating PSUM tile, drained into a
    persistent SBUF accumulator with DVE adds.
  - Poisson partials + raw diagonal dots + own norms via DVE fused
    multiply+reduce on bf16 own slices (host-cast; accumulation in f32).
  - Host does only the O(B) final combine: log of 4096-length sums, means.

All DRAM inputs are host-pre-tiled so every DMA moves 128 contiguous
multi-KB runs (one per SBUF partition): own/inp/tgt as [P, MT*D] bf16,
b^T features as [NCH*P, K*NTC] fp8 chunk-major.  inp/tgt are queued last
(only needed by the tail).  HBM traffic per core: ~13 MiB.
"""

import math
import sys

import numpy as np

sys.path.insert(0, "/opt/trn_rl_repo")

P = 128
TEMPERATURE = 0.5
EPS_POISSON = 1e-8
OSCALE = 16.0  # own-side normalized rows scaled by this before fp8 cast


class Cfg:
    def __init__(self, B=4096, D=1024, n_cores=8, ntc=1024, use_ag=False):
        self.use_ag = use_ag        # AllGather own-row norms instead of Gram
        self.B = B                  # batch
        self.D = D                  # feature dim
        self.n_cores = n_cores
        self.S = B // n_cores       # own rows per core
        self.MT = self.S // P       # own-row tiles
        self.K = D // P             # contraction subtiles
        self.KP = self.K // 2       # fp8 DoubleRow k-pairs
        self.G = B // P             # n-subtiles over the full batch
        self.NTC = min(ntc, B)      # columns per DMA chunk of b^T
        self.NCH = B // self.NTC    # chunks per feature
        self.SC = self.NTC // P     # n-subtiles per chunk
        assert B % n_cores == 0 and self.S % P == 0 and D % (2 * P) == 0
        assert B % self.NTC == 0 and self.NTC % P == 0 and self.SC % 2 == 0


def _patch_act_tables():
    """Make Bacc's act-table pass pick `natural_log_exp_and_others` for both
    Exp and Ln (they otherwise land in two different sets, and alternating
    Ln/Exp calls reload the 2.7us activation tables every tile)."""
    import functools

    import concourse.hw_specs as hw_specs

    if getattr(hw_specs, "_act_tables_patched", False):
        return
    orig = hw_specs.get_activation_tables

    @functools.cache
    def patched(module_arch):
        tabs = dict(orig(module_arch))
        names = list(tabs.keys())
        if "natural_log_exp_and_others" in tabs:
            combined = tabs["natural_log_exp_and_others"]
            for name in names:
                if name == "natural_log_exp_and_others":
                    break
                if tabs[name] & combined:
                    tabs[name] = tabs[name] - combined
        return tabs

    hw_specs.get_activation_tables = patched
    import concourse.bacc as bacc_mod

    if hasattr(bacc_mod, "get_activation_tables"):
        bacc_mod.get_activation_tables = patched
    hw_specs._act_tables_patched = True


def build_bass(cfg: Cfg):
    """Build the single-core Bass program (same program for all SPMD cores)."""
    import concourse.bacc as bacc
    import concourse.bass as bass
    import concourse.mybir as mybir
    import concourse.tile as tile
    from concourse.masks import make_identity

    _patch_act_tables()

    f32 = mybir.dt.float32
    bf16 = mybir.dt.bfloat16
    fp8 = mybir.dt.float8e4
    AF = mybir.ActivationFunctionType
    ALU = mybir.AluOpType
    DR = mybir.MatmulPerfMode.DoubleRow
    ts = bass.ts

    B, D, S, MT, K, KP, G, NTC, NCH, SC = (
        cfg.B, cfg.D, cfg.S, cfg.MT, cfg.K, cfg.KP, cfg.G, cfg.NTC, cfg.NCH, cfg.SC,
    )

    nc = bacc.Bacc(
        "TRN2",
        target_bir_lowering=False,
        debug=False,
        enable_asserts=False,
        num_devices=cfg.n_cores,
    )

    # ---- IO (all host-pre-tiled, see make_in_maps) ----
    f1o = nc.dram_tensor("f1_own", [P, MT * D], bf16, kind="ExternalInput").ap()
    f2o = nc.dram_tensor("f2_own", [P, MT * D], bf16, kind="ExternalInput").ap()
    f3o = nc.dram_tensor("f3_own", [P, MT * D], bf16, kind="ExternalInput").ap()
    inp = nc.dram_tensor("inp_own", [P, MT * D], bf16, kind="ExternalInput").ap()
    tgt = nc.dram_tensor("tgt_own", [P, MT * D], bf16, kind="ExternalInput").ap()
    f2T = nc.dram_tensor("f2T8", [NCH * P, K * NTC], fp8, kind="ExternalInput").ap()
    f3T = nc.dram_tensor("f3T8", [NCH * P, K * NTC], fp8, kind="ExternalInput").ap()

    if cfg.use_ag:
        ag_in = nc.dram_tensor("nsq_ag_in", [P, 2 * MT], f32).ap()
        ag_out = nc.dram_tensor(
            "nsq_ag_out", [cfg.n_cores * P, 2 * MT], f32,
            addr_space="Shared" if cfg.n_cores > 4 else "Local",
        ).ap()
    nsq_d = nc.dram_tensor("nsq_own", [P, 3 * MT], f32, kind="ExternalOutput").ap()
    dots_d = nc.dram_tensor("dots_own", [P, 3 * MT], f32, kind="ExternalOutput").ap()
    poi_d = nc.dram_tensor("poi", [P, 2 * MT], f32, kind="ExternalOutput").ap()
    rows_d = nc.dram_tensor("rowsums", [1, 3 * S], f32, kind="ExternalOutput").ap()
    colp_d = nc.dram_tensor("colparts", [P, 3 * G], f32, kind="ExternalOutput").ap()

    own_dram = [f1o, f2o, f3o]
    fT_dram = [f2T, f3T]
    # pairs as (pair_index, own_feature a) grouped by the full-side feature b
    pairs_of_b = [[(0, 0)], [(1, 0), (2, 1)]]  # b=f2: (f1,f2); b=f3: (f1,f3),(f2,f3)

    with tile.TileContext(nc) as tc:
        with (
            tc.tile_pool(name="const", bufs=1) as const_pool,
            tc.tile_pool(name="persist", bufs=1) as persist,
            tc.tile_pool(name="stage16", bufs=4) as stage16,
            tc.tile_pool(name="lg", bufs=2) as lgp,
            tc.tile_pool(name="junk", bufs=2) as junkp,
            tc.tile_pool(name="exps", bufs=3) as expp,
            tc.tile_pool(name="small", bufs=6) as smallp,
            tc.tile_pool(name="ps_s", bufs=3, space="PSUM") as ps_s,
            tc.tile_pool(name="ps_row", bufs=1, space="PSUM") as ps_rowp,
            tc.tile_pool(name="ps_t", bufs=2, space="PSUM") as ps_t,
        ):
            ident16 = const_pool.tile([P, P], bf16)
            make_identity(nc, ident16)
            ident32 = const_pool.tile([P, P], f32)
            make_identity(nc, ident32)
            # two ones per partition, 16B apart (DoubleRow weight APs need the
            # k-pair stride 16B-aligned)
            ones8_pad = const_pool.tile([P, 2, 16], fp8)
            nc.vector.memset(ones8_pad, 1.0)
            ones8 = ones8_pad[:, :, 0:1]
            eps_bias = const_pool.tile([P, 1], f32)
            nc.vector.memset(eps_bias, EPS_POISSON)
            ln16_bias = const_pool.tile([P, 1], f32)
            nc.vector.memset(ln16_bias, math.log(OSCALE))
            lnbt_bias = const_pool.tile([P, 1], f32)
            nc.vector.memset(lnbt_bias, -math.log(OSCALE * TEMPERATURE))

            # ---- PE warmup: ~35 junk transposes keep the PE busy from
            # t~6us so the HAM clock-gate un-throttles (4/8 -> 8/8) before
            # the real matmul stream begins (saves ~10us of half-clock).
            for w in range(36):
                wps = ps_t.tile([P, K * P], bf16, tag="tps")
                nc.tensor.transpose(wps[:, 0:P], ident16, ident16)

            # persistent state
            fT_sb = [
                [persist.tile([P, K, NTC], fp8, name=f"fT{b}_{ch}") for ch in range(NCH)]
                for b in range(2)
            ]
            own_sb = [
                persist.tile([P, MT, D], bf16, name=f"own{fi}") for fi in range(3)
            ]
            it_sb = persist.tile([P, MT, D], bf16)
            tt_sb = persist.tile([P, MT, D], bf16)
            zT_own = [persist.tile([P, K, S], fp8, name=f"zT_own{a}") for a in range(2)]
            nsq_own = persist.tile([P, 3 * MT], f32)
            dots_own = persist.tile([P, 3 * MT], f32)
            poi = persist.tile([P, 2 * MT], f32)
            scale16 = persist.tile([P, 2 * MT], f32)   # 16/||a|| for f1,f2 own
            bnsq = persist.tile([P, 2 * G], f32)       # ||b_n||^2 (fp8 data)
            bscale = persist.tile([P, 2 * G], f32)     # 1/(16*T*||b_n||)
            colp_sb = persist.tile([P, 3 * G], f32)
            rows_acc = persist.tile([1, 3 * S], f32)

            # ---- DMA dispatch: fT chunks on the scalar HWDGE queue, own
            # features on sync; inp/tgt queue last on scalar (tail-only data).
            for b in range(2):
                for ch in range(NCH):
                    nc.scalar.dma_start(
                        fT_sb[b][ch],
                        fT_dram[b][ts(ch, P), :].rearrange("p (k n) -> p k n", k=K),
                    )
            for fi in range(3):
                nc.sync.dma_start(
                    own_sb[fi], own_dram[fi].rearrange("p (t d) -> p t d", t=MT)
                )
            nc.scalar.dma_start(it_sb, inp.rearrange("p (t d) -> p t d", t=MT))
            nc.scalar.dma_start(tt_sb, tgt.rearrange("p (t d) -> p t d", t=MT))

            nc.vector.memset(rows_acc, 0.0)

            # ---- phase A: own norms (f1/f2 only - f3's in the tail), ----
            for fi in range(2):
                for t in range(MT):
                    jt = junkp.tile([P, D], bf16, tag="junk16")
                    nc.vector.scalar_tensor_tensor(
                        out=jt, in0=own_sb[fi][:, t, :], scalar=1.0,
                        in1=own_sb[fi][:, t, :],
                        op0=ALU.mult, op1=ALU.mult,
                        accum_out=nsq_own[:, fi * MT + t : fi * MT + t + 1],
                    )

            # scale16 = 16 / ||a||  (ACT: exp(-0.5*ln(nsq) + ln 16))
            lnq = smallp.tile([P, 2 * MT], f32, tag="ln_own")
            nc.scalar.activation(lnq, nsq_own[:, : 2 * MT], AF.Ln)
            nc.scalar.activation(
                scale16, lnq, AF.Exp, scale=-0.5, bias=ln16_bias[:, :]
            )

            if cfg.use_ag:
                # share f2/f3 own-row norms across cores: 4KB AllGather.
                # bnsq[lane, b*G + c*MT + t] = ag_out[c*128+lane, b*MT+t]
                nc.sync.dma_start(ag_in, nsq_own[:, MT : 3 * MT])
                nc.gpsimd.collective_compute(
                    "AllGather",
                    mybir.AluOpType.bypass,
                    [list(range(cfg.n_cores))],
                    ins=[ag_in],
                    outs=[ag_out],
                )
                nc.sync.dma_start(
                    bnsq[:].rearrange("p (b c t) -> p b c t", b=2, c=cfg.n_cores),
                    ag_out.rearrange("(c p) (b t) -> p b c t", p=P, b=2),
                )
                lnb = smallp.tile([P, 2 * G], f32, tag="lnb_all")
                nc.scalar.activation(lnb, bnsq, AF.Ln)
                nc.scalar.activation(
                    bscale, lnb, AF.Exp, scale=-0.5, bias=lnbt_bias[:, :]
                )

            # normalize own f1/f2 rows (x16, bf16) then PE-transpose; the
            # PSUM->SBUF copy casts to fp8.
            for fi in range(2):
                for t in range(MT):
                    zrow = stage16.tile([P, D], bf16, tag="zhat16")
                    nc.vector.tensor_scalar_mul(
                        zrow, own_sb[fi][:, t, :],
                        scale16[:, fi * MT + t : fi * MT + t + 1],
                    )
                    tps = ps_t.tile([P, K * P], bf16, tag="tps")
                    for k in range(K):
                        nc.tensor.transpose(tps[:, ts(k, P)], zrow[:, ts(k, P)], ident16)
                    nc.vector.tensor_copy(
                        out=zT_own[fi][:, :, ts(t, P)],
                        in_=tps[:].rearrange("p (k c) -> p k c", k=K),
                    )

            def tail_tile(t):
                # f3 norms + diag dots + poisson partials for row-tile t
                jt0 = junkp.tile([P, D], bf16, tag="junk16")
                nc.vector.scalar_tensor_tensor(
                    out=jt0, in0=own_sb[2][:, t, :], scalar=1.0,
                    in1=own_sb[2][:, t, :],
                    op0=ALU.mult, op1=ALU.mult,
                    accum_out=nsq_own[:, 2 * MT + t : 2 * MT + t + 1],
                )
                for pi, (ia, ib) in enumerate(((0, 1), (0, 2), (1, 2))):
                    jt = junkp.tile([P, D], bf16, tag="junk16")
                    nc.vector.scalar_tensor_tensor(
                        out=jt, in0=own_sb[ia][:, t, :], scalar=1.0,
                        in1=own_sb[ib][:, t, :],
                        op0=ALU.mult, op1=ALU.mult,
                        accum_out=dots_own[:, pi * MT + t : pi * MT + t + 1],
                    )
                lg = lgp.tile([P, D], f32, tag="lg")
                nc.scalar.activation(lg, it_sb[:, t, :], AF.Ln, bias=eps_bias[:, :])
                jt = junkp.tile([P, D], bf16, tag="junk16")
                nc.vector.scalar_tensor_tensor(
                    out=jt, in0=tt_sb[:, t, :], scalar=1.0, in1=lg,
                    op0=ALU.mult, op1=ALU.mult,
                    accum_out=poi[:, MT + t : MT + t + 1],
                )
                jt2 = junkp.tile([P, D], bf16, tag="junk16")
                nc.vector.tensor_scalar(
                    out=jt2, in0=it_sb[:, t, :], scalar1=1.0, scalar2=0.0,
                    op0=ALU.mult, op1=ALU.add, accum_out=poi[:, t : t + 1],
                )

            # distribute the MT tail tiles over the b=1 chunks (ACT/DVE slack)
            tail_sched = {}
            for t in range(MT):
                tail_sched.setdefault(t * NCH // MT if NCH >= MT else t % NCH, []).append(t)

            # ---- phase B: stream both b features ----
            for b in range(2):
                for ch in range(NCH):
                    if not cfg.use_ag:
                        # b-norms for this chunk: fp8 Gram diagonals
                        for s in range(SC):
                            g = ch * SC + s
                            gram = ps_t.tile([P, P], f32, tag="gram")
                            bsub = fT_sb[b][ch][:, :, ts(s, P)]
                            for j in range(KP):
                                nc.tensor.matmul(
                                    gram,
                                    bsub[:, 2 * j : 2 * j + 2, :],
                                    bsub[:, 2 * j : 2 * j + 2, :],
                                    start=(j == 0), stop=(j == KP - 1),
                                    perf_mode=DR,
                                )
                            j8 = junkp.tile([P, P], bf16, tag="junkg")
                            nc.vector.scalar_tensor_tensor(
                                out=j8, in0=gram, scalar=1.0, in1=ident32,
                                op0=ALU.mult, op1=ALU.mult,
                                accum_out=bnsq[:, b * G + g : b * G + g + 1],
                            )
                        lnb = smallp.tile([P, SC], f32, tag="lnb")
                        sl = slice(b * G + ch * SC, b * G + (ch + 1) * SC)
                        nc.scalar.activation(lnb, bnsq[:, sl], AF.Ln)
                        nc.scalar.activation(
                            bscale[:, sl], lnb, AF.Exp,
                            scale=-0.5, bias=lnbt_bias[:, :],
                        )

                    # sim matmuls + exp + row/col sums
                    for (pair, a) in pairs_of_b[b]:
                        e2 = None
                        rp = ps_rowp.tile([1, S], f32, tag="rp")
                        for s in range(SC):
                            g = ch * SC + s
                            ps = ps_s.tile([P, S], f32, tag="ps_s")
                            bsub = fT_sb[b][ch][:, :, ts(s, P)]
                            for j in range(KP):
                                nc.tensor.matmul(
                                    ps,
                                    bsub[:, 2 * j : 2 * j + 2, :],
                                    zT_own[a][:, 2 * j : 2 * j + 2, :],
                                    start=(j == 0), stop=(j == KP - 1),
                                    perf_mode=DR,
                                )
                            if s % 2 == 0:
                                e2 = expp.tile([P, 2, S], fp8, tag="exp8")
                            nc.scalar.activation(
                                e2[:, s % 2, :], ps, AF.Exp,
                                scale=bscale[:, b * G + g : b * G + g + 1],
                                accum_out=colp_sb[:, pair * G + g : pair * G + g + 1],
                            )
                            if s % 2 == 1:
                                nc.tensor.matmul(
                                    rp,
                                    ones8,
                                    e2[:, :, :],
                                    start=(s == 1),
                                    stop=(s == SC - 1),
                                    perf_mode=DR,
                                    skip_group_check=True,
                                )
                        acc = rows_acc[:, pair * S : (pair + 1) * S]
                        nc.vector.tensor_tensor(
                            out=acc, in0=rp, in1=acc, op=ALU.add
                        )
                    if b == 1:
                        for t in tail_sched.get(ch, []):
                            tail_tile(t)

            # ---- outputs ----
            nc.gpsimd.dma_start(rows_d, rows_acc)
            nc.gpsimd.dma_start(colp_d, colp_sb)
            nc.gpsimd.dma_start(nsq_d, nsq_own)
            nc.gpsimd.dma_start(dots_d, dots_own)
            nc.gpsimd.dma_start(poi_d, poi)

    nc.compile()
    return nc


def _tile_rows(a, MT):
    """[S, D] -> [P, MT*D] contiguous: row (t*128+p) -> out[p, t*D:(t+1)*D]."""
    S, D = a.shape
    return np.ascontiguousarray(
        a.reshape(MT, P, D).transpose(1, 0, 2).reshape(P, MT * D)
    )


def make_in_maps(cfg: Cfg, inputs, targets, feature1, feature2, feature3):
    import ml_dtypes

    bf16 = ml_dtypes.bfloat16
    fp8 = ml_dtypes.float8_e4m3
    K, NCH, NTC, MT = cfg.K, cfg.NCH, cfg.NTC, cfg.MT

    def prep_fT(f):
        # [B, D] f32 -> fp8 b^T pre-tiled [NCH*P, K*NTC]:
        # row (ch*P + p), col (k*NTC + n) = f[ch*NTC + n, k*P + p]
        t8 = np.ascontiguousarray(np.asarray(f, dtype=np.float32).T).astype(fp8)
        # t8: [D, B] = [K*P, NCH*NTC]
        return np.ascontiguousarray(
            t8.reshape(K, P, NCH, NTC).transpose(2, 1, 0, 3).reshape(NCH * P, K * NTC)
        )

    f2T8 = prep_fT(feature2)
    f3T8 = prep_fT(feature3)
    maps = []
    for c in range(cfg.n_cores):
        sl = slice(c * cfg.S, (c + 1) * cfg.S)
        maps.append({
            "f1_own": _tile_rows(np.asarray(feature1[sl]).astype(bf16), MT),
            "f2_own": _tile_rows(np.asarray(feature2[sl]).astype(bf16), MT),
            "f3_own": _tile_rows(np.asarray(feature3[sl]).astype(bf16), MT),
            "inp_own": _tile_rows(np.asarray(inputs[sl]).astype(bf16), MT),
            "tgt_own": _tile_rows(np.asarray(targets[sl]).astype(bf16), MT),
            "f2T8": f2T8,
            "f3T8": f3T8,
        })
    return maps


def combine_results(cfg: Cfg, per_core):
    """per_core: list of dicts with rowsums/colparts/nsq_own/dots_own/poi."""
    B, MT, S, G = cfg.B, cfg.MT, cfg.S, cfg.G
    nsq = np.zeros((3, B), np.float64)
    dots = np.zeros((3, B), np.float64)
    rowsum = np.zeros((3, B), np.float64)
    colsum = np.zeros((3, B), np.float64)
    poi_in = 0.0
    poi_tl = 0.0
    for c, r in enumerate(per_core):
        rs = np.asarray(r["rowsums"], np.float64).reshape(3, S)
        cp = np.asarray(r["colparts"], np.float64)      # [128, 3*G]
        nq = np.asarray(r["nsq_own"], np.float64)       # [128, 3*MT]
        dt_ = np.asarray(r["dots_own"], np.float64)
        po = np.asarray(r["poi"], np.float64)           # [128, 2*MT]
        for fi in range(3):
            for t in range(MT):
                rows = slice(c * S + t * P, c * S + (t + 1) * P)
                nsq[fi, rows] = nq[:, fi * MT + t]
        for pi in range(3):
            rowsum[pi, c * S : (c + 1) * S] = rs[pi]
            for t in range(MT):
                rows = slice(c * S + t * P, c * S + (t + 1) * P)
                dots[pi, rows] = dt_[:, pi * MT + t]
            # colparts: n = g*128 + lane
            colsum[pi] += cp[:, pi * G : (pi + 1) * G].T.reshape(-1)
        poi_in += po[:, :MT].sum()
        poi_tl += po[:, MT:].sum()

    na = np.sqrt(nsq)  # [3, B]
    pairs = ((0, 1), (0, 2), (1, 2))
    closs = 0.0
    for pi, (ia, ib) in enumerate(pairs):
        simdiag = dots[pi] / (na[ia] * na[ib])
        loss_i = np.mean(np.log(rowsum[pi]) - simdiag / TEMPERATURE)
        loss_j = np.mean(np.log(colsum[pi]) - simdiag / TEMPERATURE)
        closs += 0.5 * (loss_i + loss_j)
    closs /= 3.0
    p_loss = (poi_in - poi_tl) / (cfg.B * cfg.D)
    total = p_loss + closs
    return (
        np.float32(total),
        np.float32(p_loss),
        np.float32(closs),
    )


_CACHE = {}


def _get_compiled(cfg: Cfg):
    key = (cfg.B, cfg.D, cfg.n_cores, cfg.NTC)
    if key not in _CACHE:
        _CACHE[key] = build_bass(cfg)
    return _CACHE[key]


def kernel(inputs, targets, feature1, feature2, feature3):
    from concourse.bass_utils import run_bass_kernel_spmd

    cfg = Cfg(B=inputs.shape[0], D=inputs.shape[1], n_cores=8)
    nc = _get_compiled(cfg)
    in_maps = make_in_maps(cfg, inputs, targets, feature1, feature2, feature3)
    res = run_bass_kernel_spmd(nc, in_maps, core_ids=list(range(cfg.n_cores)))
    return combine_results(cfg, res.results)


if __name__ == "__main__":
    # smoke test on hardware with full shapes
    rng = np.random.default_rng(0)
    B, D = 4096, 1024
    ins = {
        "inputs": rng.random((B, D), np.float32),
        "targets": rng.random((B, D), np.float32),
        "feature1": rng.standard_normal((B, D), np.float32),
        "feature2": rng.standard_normal((B, D), np.float32),
        "feature3": rng.standard_normal((B, D), np.float32),
    }
    out = kernel(**ins)
    print(out)


# revision 24
# speedup vs baseline: 1.0102x; 1.0102x over previous
"""Trainium2 Bass kernel for nn_CombinedLoss (Poisson + 3-way pairwise CLIP loss).

Strategy (8 NeuronCores, SPMD, no collectives), v3 "flipped orientation":
  - Row-shard the batch: core c owns rows [c*512, (c+1)*512) of every tensor.
  - For each pair (a,b) in {(1,2),(1,3),(2,3)} each core computes the FULL
    column strip S_ab^T[n, m] = <b_n_raw, zhat_a_m> for all 4096 n and its own
    512 m, using fp8(e4m3) DoubleRow matmuls (2x PE throughput, fp32 PSUM):
      lhsT (stationary) = raw fp8 b^T tiles, DMA'd directly from a
             host-side transposed+fp8-cast+pre-tiled copy of the full feature
             (no on-device normalization / transpose / scratch roundtrip);
      rhs  (moving)     = own rows of a, normalized (x16) to fp8 and
             PE-transposed on-chip.
  - The missing 1/||b_n|| normalization is per-PSUM-PARTITION in this
    orientation, so it folds into the Exp's per-partition scale:
    exp(S_raw[n,m] / (16 * T * ||b_n||)).  ||b_n||^2 for all n comes from
    fp8 Gram-diagonal matmuls on the already-loaded b^T tiles (diag extracted
    with a DVE multiply-by-identity + free-dim accumulate).
  - Column sums over own m (partial, host-combined) come free via the Exp's
    accum_out.  Row sums over all n are ones-vector DoubleRow matmuls
    accumulated per (pair, chunk) in a rotating PSUM tile, drained into a
    persistent SBUF accumulator with DVE adds.
  - Poisson partials + raw diagonal dots + own norms via DVE fused
    multiply+reduce on bf16 own slices (host-cast; accumulation in f32).
  - Host does only the O(B) final combine: log of 4096-length sums, means.

All DRAM inputs are host-pre-tiled so every DMA moves 128 contiguous
multi-KB runs (one per SBUF partition): own/inp/tgt as [P, MT*D] bf16,
b^T features as [NCH*P, K*NTC] fp8 chunk-major.  inp/tgt are queued last
(only needed by the tail).  HBM traffic per core: ~13 MiB.
"""

import math
import sys

import numpy as np

sys.path.insert(0, "/opt/trn_rl_repo")

P = 128
TEMPERATURE = 0.5
EPS_POISSON = 1e-8
OSCALE = 16.0  # own-side normalized rows scaled by this before fp8 cast


class Cfg:
    def __init__(self, B=4096, D=1024, n_cores=8, ntc=1024, use_ag=False):
        self.use_ag = use_ag        # AllGather own-row norms instead of Gram
        self.B = B                  # batch
        self.D = D                  # feature dim
        self.n_cores = n_cores
        self.S = B // n_cores       # own rows per core
        self.MT = self.S // P       # own-row tiles
        self.K = D // P             # contraction subtiles
        self.KP = self.K // 2       # fp8 DoubleRow k-pairs
        self.G = B // P             # n-subtiles over the full batch
        self.NTC = min(ntc, B)      # columns per DMA chunk of b^T
        self.NCH = B // self.NTC    # chunks per feature
        self.SC = self.NTC // P     # n-subtiles per chunk
        assert B % n_cores == 0 and self.S % P == 0 and D % (2 * P) == 0
        assert B % self.NTC == 0 and self.NTC % P == 0 and self.SC % 2 == 0


def _patch_act_tables():
    """Make Bacc's act-table pass pick `natural_log_exp_and_others` for both
    Exp and Ln (they otherwise land in two different sets, and alternating
    Ln/Exp calls reload the 2.7us activation tables every tile)."""
    import functools

    import concourse.hw_specs as hw_specs

    if getattr(hw_specs, "_act_tables_patched", False):
        return
    orig = hw_specs.get_activation_tables

    @functools.cache
    def patched(module_arch):
        tabs = dict(orig(module_arch))
        names = list(tabs.keys())
        if "natural_log_exp_and_others" in tabs:
            combined = tabs["natural_log_exp_and_others"]
            for name in names:
                if name == "natural_log_exp_and_others":
                    break
                if tabs[name] & combined:
                    tabs[name] = tabs[name] - combined
        return tabs

    hw_specs.get_activation_tables = patched
    import concourse.bacc as bacc_mod

    if hasattr(bacc_mod, "get_activation_tables"):
        bacc_mod.get_activation_tables = patched
    hw_specs._act_tables_patched = True


def build_bass(cfg: Cfg):
    """Build the single-core Bass program (same program for all SPMD cores)."""
    import concourse.bacc as bacc
    import concourse.bass as bass
    import concourse.mybir as mybir
    import concourse.tile as tile
    from concourse.masks import make_identity

    _patch_act_tables()

    f32 = mybir.dt.float32
    bf16 = mybir.dt.bfloat16
    fp8 = mybir.dt.float8e4
    AF = mybir.ActivationFunctionType
    ALU = mybir.AluOpType
    DR = mybir.MatmulPerfMode.DoubleRow
    ts = bass.ts

    B, D, S, MT, K, KP, G, NTC, NCH, SC = (
        cfg.B, cfg.D, cfg.S, cfg.MT, cfg.K, cfg.KP, cfg.G, cfg.NTC, cfg.NCH, cfg.SC,
    )

    nc = bacc.Bacc(
        "TRN2",
        target_bir_lowering=False,
        debug=False,
        enable_asserts=False,
        num_devices=cfg.n_cores,
    )

    # ---- IO (all host-pre-tiled, see make_in_maps) ----
    f1o = nc.dram_tensor("f1_own", [P, MT * D], bf16, kind="ExternalInput").ap()
    f2o = nc.dram_tensor("f2_own", [P, MT * D], bf16, kind="ExternalInput").ap()
    f3o = nc.dram_tensor("f3_own", [P, MT * D], bf16, kind="ExternalInput").ap()
    inp = nc.dram_tensor("inp_own", [P, MT * D], bf16, kind="ExternalInput").ap()
    tgt = nc.dram_tensor("tgt_own", [P, MT * D], bf16, kind="ExternalInput").ap()
    f2T = nc.dram_tensor("f2T8", [NCH * P, K * NTC], fp8, kind="ExternalInput").ap()
    f3T = nc.dram_tensor("f3T8", [NCH * P, K * NTC], fp8, kind="ExternalInput").ap()

    if cfg.use_ag:
        ag_in = nc.dram_tensor("nsq_ag_in", [P, 2 * MT], f32).ap()
        ag_out = nc.dram_tensor(
            "nsq_ag_out", [cfg.n_cores * P, 2 * MT], f32,
            addr_space="Shared" if cfg.n_cores > 4 else "Local",
        ).ap()
    nsq_d = nc.dram_tensor("nsq_own", [P, 3 * MT], f32, kind="ExternalOutput").ap()
    dots_d = nc.dram_tensor("dots_own", [P, 3 * MT], f32, kind="ExternalOutput").ap()
    poi_d = nc.dram_tensor("poi", [P, 2 * MT], f32, kind="ExternalOutput").ap()
    rows_d = nc.dram_tensor("rowsums", [1, 3 * S], f32, kind="ExternalOutput").ap()
    colp_d = nc.dram_tensor("colparts", [P, 3 * G], f32, kind="ExternalOutput").ap()

    own_dram = [f1o, f2o, f3o]
    fT_dram = [f2T, f3T]
    # pairs as (pair_index, own_feature a) grouped by the full-side feature b
    pairs_of_b = [[(0, 0)], [(1, 0), (2, 1)]]  # b=f2: (f1,f2); b=f3: (f1,f3),(f2,f3)

    with tile.TileContext(nc) as tc:
        with (
            tc.tile_pool(name="const", bufs=1) as const_pool,
            tc.tile_pool(name="persist", bufs=1) as persist,
            tc.tile_pool(name="stage16", bufs=4) as stage16,
            tc.tile_pool(name="lg", bufs=2) as lgp,
            tc.tile_pool(name="junk", bufs=2) as junkp,
            tc.tile_pool(name="exps", bufs=3) as expp,
            tc.tile_pool(name="small", bufs=6) as smallp,
            tc.tile_pool(name="ps_s", bufs=3, space="PSUM") as ps_s,
            tc.tile_pool(name="ps_row", bufs=1, space="PSUM") as ps_rowp,
            tc.tile_pool(name="ps_t", bufs=2, space="PSUM") as ps_t,
            tc.tile_pool(name="ps_g", bufs=2, space="PSUM") as ps_g,
        ):
            ident16 = const_pool.tile([P, P], bf16)
            make_identity(nc, ident16)
            ident32 = const_pool.tile([P, P], f32)
            make_identity(nc, ident32)
            # two ones per partition, 16B apart (DoubleRow weight APs need the
            # k-pair stride 16B-aligned)
            ones8_pad = const_pool.tile([P, 2, 16], fp8)
            nc.vector.memset(ones8_pad, 1.0)
            ones8 = ones8_pad[:, :, 0:1]
            eps_bias = const_pool.tile([P, 1], f32)
            nc.vector.memset(eps_bias, EPS_POISSON)
            ln16_bias = const_pool.tile([P, 1], f32)
            nc.vector.memset(ln16_bias, math.log(OSCALE))
            lnbt_bias = const_pool.tile([P, 1], f32)
            nc.vector.memset(lnbt_bias, -math.log(OSCALE * TEMPERATURE))

            # persistent state
            fT_sb = [
                [persist.tile([P, K, NTC], fp8, name=f"fT{b}_{ch}") for ch in range(NCH)]
                for b in range(2)
            ]
            own_sb = [
                persist.tile([P, MT, D], bf16, name=f"own{fi}") for fi in range(3)
            ]
            it_sb = persist.tile([P, MT, D], bf16)
            tt_sb = persist.tile([P, MT, D], bf16)
            zT_own = [persist.tile([P, K, S], fp8, name=f"zT_own{a}") for a in range(2)]
            nsq_own = persist.tile([P, 3 * MT], f32)
            dots_own = persist.tile([P, 3 * MT], f32)
            poi = persist.tile([P, 2 * MT], f32)
            scale16 = persist.tile([P, 2 * MT], f32)   # 16/||a|| for f1,f2 own
            bnsq = persist.tile([P, 2 * G], f32)       # ||b_n||^2 (fp8 data)
            bscale = persist.tile([P, 2 * G], f32)     # 1/(16*T*||b_n||)
            colp_sb = persist.tile([P, 3 * G], f32)
            rows_acc = persist.tile([1, 3 * S], f32)

            # ---- DMA dispatch: fT chunks on the scalar HWDGE queue, own
            # features on sync; inp/tgt queue last on scalar (tail-only data).
            for b in (1, 0):
                for ch in range(NCH):
                    nc.scalar.dma_start(
                        fT_sb[b][ch],
                        fT_dram[b][ts(ch, P), :].rearrange("p (k n) -> p k n", k=K),
                    )
            for fi in range(3):
                nc.sync.dma_start(
                    own_sb[fi], own_dram[fi].rearrange("p (t d) -> p t d", t=MT)
                )
            nc.scalar.dma_start(it_sb, inp.rearrange("p (t d) -> p t d", t=MT))
            nc.scalar.dma_start(tt_sb, tgt.rearrange("p (t d) -> p t d", t=MT))

            nc.vector.memset(rows_acc, 0.0)

            # ---- phase A: own norms (f1/f2 only - f3's in the tail), ----
            for fi in range(2):
                for t in range(MT):
                    jt = junkp.tile([P, D], bf16, tag="junk16")
                    nc.vector.scalar_tensor_tensor(
                        out=jt, in0=own_sb[fi][:, t, :], scalar=1.0,
                        in1=own_sb[fi][:, t, :],
                        op0=ALU.mult, op1=ALU.mult,
                        accum_out=nsq_own[:, fi * MT + t : fi * MT + t + 1],
                    )

            # scale16 = 16 / ||a||  (ACT: exp(-0.5*ln(nsq) + ln 16))
            lnq = smallp.tile([P, 2 * MT], f32, tag="ln_own")
            nc.scalar.activation(lnq, nsq_own[:, : 2 * MT], AF.Ln)
            nc.scalar.activation(
                scale16, lnq, AF.Exp, scale=-0.5, bias=ln16_bias[:, :]
            )

            if cfg.use_ag:
                # share f2/f3 own-row norms across cores: 4KB AllGather.
                # bnsq[lane, b*G + c*MT + t] = ag_out[c*128+lane, b*MT+t]
                nc.sync.dma_start(ag_in, nsq_own[:, MT : 3 * MT])
                nc.gpsimd.collective_compute(
                    "AllGather",
                    mybir.AluOpType.bypass,
                    [list(range(cfg.n_cores))],
                    ins=[ag_in],
                    outs=[ag_out],
                )
                nc.sync.dma_start(
                    bnsq[:].rearrange("p (b c t) -> p b c t", b=2, c=cfg.n_cores),
                    ag_out.rearrange("(c p) (b t) -> p b c t", p=P, b=2),
                )
                lnb = smallp.tile([P, 2 * G], f32, tag="lnb_all")
                nc.scalar.activation(lnb, bnsq, AF.Ln)
                nc.scalar.activation(
                    bscale, lnb, AF.Exp, scale=-0.5, bias=lnbt_bias[:, :]
                )

            # fused normalize+transpose: one PE matmul per k-slice against
            # diag(16/||row||) (rows of identity scaled per-partition), so the
            # DVE never touches the D-wide data and the transposed result is
            # already normalized.  PSUM f32 -> fp8 in the copy out.
            KH = min(K, 4)  # k-slices per PSUM half-tile
            for fi in range(2):
                for t in range(MT):
                    diag = stage16.tile([P, P], bf16, tag="diag")
                    nc.vector.tensor_scalar_mul(
                        diag, ident16, scale16[:, fi * MT + t : fi * MT + t + 1]
                    )
                    for k0 in range(0, K, KH):
                        kw = min(KH, K - k0)
                        tps = ps_t.tile([P, KH * P], f32, tag="tps")
                        for k in range(kw):
                            nc.tensor.matmul(
                                tps[:, ts(k, P)],
                                own_sb[fi][:, t, ts(k0 + k, P)],
                                diag,
                                start=True, stop=True,
                            )
                        nc.vector.tensor_copy(
                            out=zT_own[fi][:, k0 : k0 + kw, ts(t, P)],
                            in_=tps[:, : kw * P].rearrange("p (k c) -> p k c", k=kw),
                        )

            def tail_tile(t):
                # f3 norms + diag dots + poisson partials for row-tile t
                jt0 = junkp.tile([P, D], bf16, tag="junk16")
                nc.vector.scalar_tensor_tensor(
                    out=jt0, in0=own_sb[2][:, t, :], scalar=1.0,
                    in1=own_sb[2][:, t, :],
                    op0=ALU.mult, op1=ALU.mult,
                    accum_out=nsq_own[:, 2 * MT + t : 2 * MT + t + 1],
                )
                for pi, (ia, ib) in enumerate(((0, 1), (0, 2), (1, 2))):
                    jt = junkp.tile([P, D], bf16, tag="junk16")
                    nc.vector.scalar_tensor_tensor(
                        out=jt, in0=own_sb[ia][:, t, :], scalar=1.0,
                        in1=own_sb[ib][:, t, :],
                        op0=ALU.mult, op1=ALU.mult,
                        accum_out=dots_own[:, pi * MT + t : pi * MT + t + 1],
                    )
                lg = lgp.tile([P, D], f32, tag="lg")
                nc.scalar.activation(lg, it_sb[:, t, :], AF.Ln, bias=eps_bias[:, :])
                jt = junkp.tile([P, D], bf16, tag="junk16")
                nc.vector.scalar_tensor_tensor(
                    out=jt, in0=tt_sb[:, t, :], scalar=1.0, in1=lg,
                    op0=ALU.mult, op1=ALU.mult,
                    accum_out=poi[:, MT + t : MT + t + 1],
                )
                jt2 = junkp.tile([P, D], bf16, tag="junk16")
                nc.vector.tensor_scalar(
                    out=jt2, in0=it_sb[:, t, :], scalar1=1.0, scalar2=0.0,
                    op0=ALU.mult, op1=ALU.add, accum_out=poi[:, t : t + 1],
                )

            # distribute the MT tail tiles over the b=1 chunks (ACT/DVE slack)
            tail_sched = {}
            for t in range(MT):
                tail_sched.setdefault(t * NCH // MT if NCH >= MT else t % NCH, []).append(t)

            # ---- phase B: stream both b features (f3 first: its 2 pairs
            # give the scheduler more work early; f2's single pair drains
            # faster at the end) ----
            for b in (1, 0):
                for ch in range(NCH):
                    if not cfg.use_ag:
                        # b-norms for this chunk: fp8 Gram diagonals
                        for s in range(SC):
                            g = ch * SC + s
                            gram = ps_g.tile([P, P], f32, tag="gram")
                            bsub = fT_sb[b][ch][:, :, ts(s, P)]
                            for j in range(KP):
                                nc.tensor.matmul(
                                    gram,
                                    bsub[:, 2 * j : 2 * j + 2, :],
                                    bsub[:, 2 * j : 2 * j + 2, :],
                                    start=(j == 0), stop=(j == KP - 1),
                                    perf_mode=DR,
                                )
                            j8 = junkp.tile([P, P], bf16, tag="junkg")
                            nc.vector.scalar_tensor_tensor(
                                out=j8, in0=gram, scalar=1.0, in1=ident32,
                                op0=ALU.mult, op1=ALU.mult,
                                accum_out=bnsq[:, b * G + g : b * G + g + 1],
                            )
                        lnb = smallp.tile([P, SC], f32, tag="lnb")
                        sl = slice(b * G + ch * SC, b * G + (ch + 1) * SC)
                        nc.scalar.activation(lnb, bnsq[:, sl], AF.Ln)
                        nc.scalar.activation(
                            bscale[:, sl], lnb, AF.Exp,
                            scale=-0.5, bias=lnbt_bias[:, :],
                        )

                    # sim matmuls + exp + row/col sums
                    for (pair, a) in pairs_of_b[b]:
                        e2 = None
                        rp = ps_rowp.tile([1, S], f32, tag="rp")
                        for s in range(SC):
                            g = ch * SC + s
                            ps = ps_s.tile([P, S], f32, tag="ps_s")
                            bsub = fT_sb[b][ch][:, :, ts(s, P)]
                            for j in range(KP):
                                nc.tensor.matmul(
                                    ps,
                                    bsub[:, 2 * j : 2 * j + 2, :],
                                    zT_own[a][:, 2 * j : 2 * j + 2, :],
                                    start=(j == 0), stop=(j == KP - 1),
                                    perf_mode=DR,
                                )
                            if s % 2 == 0:
                                e2 = expp.tile([P, 2, S], fp8, tag="exp8")
                            nc.scalar.activation(
                                e2[:, s % 2, :], ps, AF.Exp,
                                scale=bscale[:, b * G + g : b * G + g + 1],
                                accum_out=colp_sb[:, pair * G + g : pair * G + g + 1],
                            )
                            if s % 2 == 1:
                                nc.tensor.matmul(
                                    rp,
                                    ones8,
                                    e2[:, :, :],
                                    start=(s == 1),
                                    stop=(s == SC - 1),
                                    perf_mode=DR,
                                    skip_group_check=True,
                                )
                        acc = rows_acc[:, pair * S : (pair + 1) * S]
                        nc.vector.tensor_tensor(
                            out=acc, in0=rp, in1=acc, op=ALU.add
                        )
                    if b == 0:
                        for t in tail_sched.get(ch, []):
                            tail_tile(t)

            # ---- outputs ----
            nc.gpsimd.dma_start(rows_d, rows_acc)
            nc.gpsimd.dma_start(colp_d, colp_sb)
            nc.gpsimd.dma_start(nsq_d, nsq_own)
            nc.gpsimd.dma_start(dots_d, dots_own)
            nc.gpsimd.dma_start(poi_d, poi)

    nc.compile()
    return nc


def _tile_rows(a, MT):
    """[S, D] -> [P, MT*D] contiguous: row (t*128+p) -> out[p, t*D:(t+1)*D]."""
    S, D = a.shape
    return np.ascontiguousarray(
        a.reshape(MT, P, D).transpose(1, 0, 2).reshape(P, MT * D)
    )


def make_in_maps(cfg: Cfg, inputs, targets, feature1, feature2, feature3):
    import ml_dtypes

    bf16 = ml_dtypes.bfloat16
    fp8 = ml_dtypes.float8_e4m3
    K, NCH, NTC, MT = cfg.K, cfg.NCH, cfg.NTC, cfg.MT

    def prep_fT(f):
        # [B, D] f32 -> fp8 b^T pre-tiled [NCH*P, K*NTC]:
        # row (ch*P + p), col (k*NTC + n) = f[ch*NTC + n, k*P + p]
        t8 = np.ascontiguousarray(np.asarray(f, dtype=np.float32).T).astype(fp8)
        # t8: [D, B] = [K*P, NCH*NTC]
        return np.ascontiguousarray(
            t8.reshape(K, P, NCH, NTC).transpose(2, 1, 0, 3).reshape(NCH * P, K * NTC)
        )

    f2T8 = prep_fT(feature2)
    f3T8 = prep_fT(feature3)
    maps = []
    for c in range(cfg.n_cores):
        sl = slice(c * cfg.S, (c + 1) * cfg.S)
        maps.append({
            "f1_own": _tile_rows(np.asarray(feature1[sl]).astype(bf16), MT),
            "f2_own": _tile_rows(np.asarray(feature2[sl]).astype(bf16), MT),
            "f3_own": _tile_rows(np.asarray(feature3[sl]).astype(bf16), MT),
            "inp_own": _tile_rows(np.asarray(inputs[sl]).astype(bf16), MT),
            "tgt_own": _tile_rows(np.asarray(targets[sl]).astype(bf16), MT),
            "f2T8": f2T8,
            "f3T8": f3T8,
        })
    return maps


def combine_results(cfg: Cfg, per_core):
    """per_core: list of dicts with rowsums/colparts/nsq_own/dots_own/poi."""
    B, MT, S, G = cfg.B, cfg.MT, cfg.S, cfg.G
    nsq = np.zeros((3, B), np.float64)
    dots = np.zeros((3, B), np.float64)
    rowsum = np.zeros((3, B), np.float64)
    colsum = np.zeros((3, B), np.float64)
    poi_in = 0.0
    poi_tl = 0.0
    for c, r in enumerate(per_core):
        rs = np.asarray(r["rowsums"], np.float64).reshape(3, S)
        cp = np.asarray(r["colparts"], np.float64)      # [128, 3*G]
        nq = np.asarray(r["nsq_own"], np.float64)       # [128, 3*MT]
        dt_ = np.asarray(r["dots_own"], np.float64)
        po = np.asarray(r["poi"], np.float64)           # [128, 2*MT]
        for fi in range(3):
            for t in range(MT):
                rows = slice(c * S + t * P, c * S + (t + 1) * P)
                nsq[fi, rows] = nq[:, fi * MT + t]
        for pi in range(3):
            rowsum[pi, c * S : (c + 1) * S] = rs[pi]
            for t in range(MT):
                rows = slice(c * S + t * P, c * S + (t + 1) * P)
                dots[pi, rows] = dt_[:, pi * MT + t]
            # colparts: n = g*128 + lane
            colsum[pi] += cp[:, pi * G : (pi + 1) * G].T.reshape(-1)
        poi_in += po[:, :MT].sum()
        poi_tl += po[:, MT:].sum()

    na = np.sqrt(nsq)  # [3, B]
    pairs = ((0, 1), (0, 2), (1, 2))
    closs = 0.0
    for pi, (ia, ib) in enumerate(pairs):
        simdiag = dots[pi] / (na[ia] * na[ib])
        loss_i = np.mean(np.log(rowsum[pi]) - simdiag / TEMPERATURE)
        loss_j = np.mean(np.log(colsum[pi]) - simdiag / TEMPERATURE)
        closs += 0.5 * (loss_i + loss_j)
    closs /= 3.0
    p_loss = (poi_in - poi_tl) / (cfg.B * cfg.D)
    total = p_loss + closs
    return (
        np.float32(total),
        np.float32(p_loss),
        np.float32(closs),
    )


_CACHE = {}


def _get_compiled(cfg: Cfg):
    key = (cfg.B, cfg.D, cfg.n_cores, cfg.NTC)
    if key not in _CACHE:
        _CACHE[key] = build_bass(cfg)
    return _CACHE[key]


def kernel(inputs, targets, feature1, feature2, feature3):
    from concourse.bass_utils import run_bass_kernel_spmd

    cfg = Cfg(B=inputs.shape[0], D=inputs.shape[1], n_cores=8)
    nc = _get_compiled(cfg)
    in_maps = make_in_maps(cfg, inputs, targets, feature1, feature2, feature3)
    res = run_bass_kernel_spmd(nc, in_maps, core_ids=list(range(cfg.n_cores)))
    return combine_results(cfg, res.results)


if __name__ == "__main__":
    # smoke test on hardware with full shapes
    rng = np.random.default_rng(0)
    B, D = 4096, 1024
    ins = {
        "inputs": rng.random((B, D), np.float32),
        "targets": rng.random((B, D), np.float32),
        "feature1": rng.standard_normal((B, D), np.float32),
        "feature2": rng.standard_normal((B, D), np.float32),
        "feature3": rng.standard_normal((B, D), np.float32),
    }
    out = kernel(**ins)
    print(out)


# revision 25
# speedup vs baseline: 1.0262x; 1.0158x over previous
"""Trainium2 Bass kernel for nn_CombinedLoss (Poisson + 3-way pairwise CLIP loss).

Strategy (8 NeuronCores, SPMD, no collectives), v3 "flipped orientation":
  - Row-shard the batch: core c owns rows [c*512, (c+1)*512) of every tensor.
  - For each pair (a,b) in {(1,2),(1,3),(2,3)} each core computes the FULL
    column strip S_ab^T[n, m] = <b_n_raw, zhat_a_m> for all 4096 n and its own
    512 m, using fp8(e4m3) DoubleRow matmuls (2x PE throughput, fp32 PSUM):
      lhsT (stationary) = raw fp8 b^T tiles, DMA'd directly from a
             host-side transposed+fp8-cast+pre-tiled copy of the full feature
             (no on-device normalization / transpose / scratch roundtrip);
      rhs  (moving)     = own rows of a, normalized (x16) to fp8 and
             PE-transposed on-chip.
  - The missing 1/||b_n|| normalization is per-PSUM-PARTITION in this
    orientation, so it folds into the Exp's per-partition scale:
    exp(S_raw[n,m] / (16 * T * ||b_n||)).  ||b_n||^2 for all n comes from
    fp8 Gram-diagonal matmuls on the already-loaded b^T tiles (diag extracted
    with a DVE multiply-by-identity + free-dim accumulate).
  - Column sums over own m (partial, host-combined) come free via the Exp's
    accum_out.  Row sums over all n are ones-vector DoubleRow matmuls
    accumulated per (pair, chunk) in a rotating PSUM tile, drained into a
    persistent SBUF accumulator with DVE adds.
  - Poisson partials + raw diagonal dots + own norms via DVE fused
    multiply+reduce on bf16 own slices (host-cast; accumulation in f32).
  - Host does only the O(B) final combine: log of 4096-length sums, means.

All DRAM inputs are host-pre-tiled so every DMA moves 128 contiguous
multi-KB runs (one per SBUF partition): own/inp/tgt as [P, MT*D] bf16,
b^T features as [NCH*P, K*NTC] fp8 chunk-major.  inp/tgt are queued last
(only needed by the tail).  HBM traffic per core: ~13 MiB.
"""

import math
import sys

import numpy as np

sys.path.insert(0, "/opt/trn_rl_repo")

P = 128
TEMPERATURE = 0.5
EPS_POISSON = 1e-8
OSCALE = 16.0  # own-side normalized rows scaled by this before fp8 cast


class Cfg:
    def __init__(self, B=4096, D=1024, n_cores=8, ntc=1024, use_ag=False):
        self.use_ag = use_ag        # AllGather own-row norms instead of Gram
        self.B = B                  # batch
        self.D = D                  # feature dim
        self.n_cores = n_cores
        self.S = B // n_cores       # own rows per core
        self.MT = self.S // P       # own-row tiles
        self.K = D // P             # contraction subtiles
        self.KP = self.K // 2       # fp8 DoubleRow k-pairs
        self.G = B // P             # n-subtiles over the full batch
        self.NTC = min(ntc, B)      # columns per DMA chunk of b^T
        self.NCH = B // self.NTC    # chunks per feature
        self.SC = self.NTC // P     # n-subtiles per chunk
        assert B % n_cores == 0 and self.S % P == 0 and D % (2 * P) == 0
        assert B % self.NTC == 0 and self.NTC % P == 0 and self.SC % 2 == 0


def _patch_act_tables():
    """Make Bacc's act-table pass pick `natural_log_exp_and_others` for both
    Exp and Ln (they otherwise land in two different sets, and alternating
    Ln/Exp calls reload the 2.7us activation tables every tile)."""
    import functools

    import concourse.hw_specs as hw_specs

    if getattr(hw_specs, "_act_tables_patched", False):
        return
    orig = hw_specs.get_activation_tables

    @functools.cache
    def patched(module_arch):
        tabs = dict(orig(module_arch))
        names = list(tabs.keys())
        if "natural_log_exp_and_others" in tabs:
            combined = tabs["natural_log_exp_and_others"]
            for name in names:
                if name == "natural_log_exp_and_others":
                    break
                if tabs[name] & combined:
                    tabs[name] = tabs[name] - combined
        return tabs

    hw_specs.get_activation_tables = patched
    import concourse.bacc as bacc_mod

    if hasattr(bacc_mod, "get_activation_tables"):
        bacc_mod.get_activation_tables = patched
    hw_specs._act_tables_patched = True


def build_bass(cfg: Cfg):
    """Build the single-core Bass program (same program for all SPMD cores)."""
    import concourse.bacc as bacc
    import concourse.bass as bass
    import concourse.mybir as mybir
    import concourse.tile as tile
    from concourse.masks import make_identity

    _patch_act_tables()

    f32 = mybir.dt.float32
    bf16 = mybir.dt.bfloat16
    fp8 = mybir.dt.float8e4
    AF = mybir.ActivationFunctionType
    ALU = mybir.AluOpType
    DR = mybir.MatmulPerfMode.DoubleRow
    ts = bass.ts

    B, D, S, MT, K, KP, G, NTC, NCH, SC = (
        cfg.B, cfg.D, cfg.S, cfg.MT, cfg.K, cfg.KP, cfg.G, cfg.NTC, cfg.NCH, cfg.SC,
    )

    nc = bacc.Bacc(
        "TRN2",
        target_bir_lowering=False,
        debug=False,
        enable_asserts=False,
        num_devices=cfg.n_cores,
    )

    # ---- IO (all host-pre-tiled, see make_in_maps) ----
    f1o = nc.dram_tensor("f1_own", [P, MT * D], bf16, kind="ExternalInput").ap()
    f2o = nc.dram_tensor("f2_own", [P, MT * D], bf16, kind="ExternalInput").ap()
    f3o = nc.dram_tensor("f3_own", [P, MT * D], bf16, kind="ExternalInput").ap()
    inp = nc.dram_tensor("inp_own", [P, MT * D], bf16, kind="ExternalInput").ap()
    tgt = nc.dram_tensor("tgt_own", [P, MT * D], bf16, kind="ExternalInput").ap()
    f2T = nc.dram_tensor("f2T8", [NCH * P, K * NTC], fp8, kind="ExternalInput").ap()
    f3T = nc.dram_tensor("f3T8", [NCH * P, K * NTC], fp8, kind="ExternalInput").ap()

    if cfg.use_ag:
        ag_in = nc.dram_tensor("nsq_ag_in", [P, 2 * MT], f32).ap()
        ag_out = nc.dram_tensor(
            "nsq_ag_out", [cfg.n_cores * P, 2 * MT], f32,
            addr_space="Shared" if cfg.n_cores > 4 else "Local",
        ).ap()
    NSTAT = 3 * G + 8 * MT
    stats_d = nc.dram_tensor("stats", [P, NSTAT], f32, kind="ExternalOutput").ap()
    rows_d = nc.dram_tensor("rowsums", [1, 3 * S], f32, kind="ExternalOutput").ap()

    own_dram = [f1o, f2o, f3o]
    fT_dram = [f2T, f3T]
    # pairs as (pair_index, own_feature a) grouped by the full-side feature b
    pairs_of_b = [[(0, 0)], [(1, 0), (2, 1)]]  # b=f2: (f1,f2); b=f3: (f1,f3),(f2,f3)

    with tile.TileContext(nc) as tc:
        with (
            tc.tile_pool(name="const", bufs=1) as const_pool,
            tc.tile_pool(name="persist", bufs=1) as persist,
            tc.tile_pool(name="stage16", bufs=4) as stage16,
            tc.tile_pool(name="lg", bufs=2) as lgp,
            tc.tile_pool(name="junk", bufs=2) as junkp,
            tc.tile_pool(name="exps", bufs=3) as expp,
            tc.tile_pool(name="small", bufs=6) as smallp,
            tc.tile_pool(name="ps_s", bufs=3, space="PSUM") as ps_s,
            tc.tile_pool(name="ps_row", bufs=1, space="PSUM") as ps_rowp,
            tc.tile_pool(name="ps_t", bufs=2, space="PSUM") as ps_t,
            tc.tile_pool(name="ps_g", bufs=2, space="PSUM") as ps_g,
        ):
            ident16 = const_pool.tile([P, P], bf16)
            make_identity(nc, ident16)
            ident32 = const_pool.tile([P, P], f32)
            make_identity(nc, ident32)
            # two ones per partition, 16B apart (DoubleRow weight APs need the
            # k-pair stride 16B-aligned)
            ones8_pad = const_pool.tile([P, 2, 16], fp8)
            nc.vector.memset(ones8_pad, 1.0)
            ones8 = ones8_pad[:, :, 0:1]
            eps_bias = const_pool.tile([P, 1], f32)
            nc.vector.memset(eps_bias, EPS_POISSON)
            ln16_bias = const_pool.tile([P, 1], f32)
            nc.vector.memset(ln16_bias, math.log(OSCALE))
            lnbt_bias = const_pool.tile([P, 1], f32)
            nc.vector.memset(lnbt_bias, -math.log(OSCALE * TEMPERATURE))

            # persistent state
            fT_sb = [
                [persist.tile([P, K, NTC], fp8, name=f"fT{b}_{ch}") for ch in range(NCH)]
                for b in range(2)
            ]
            own_sb = [
                persist.tile([P, MT, D], bf16, name=f"own{fi}") for fi in range(3)
            ]
            it_sb = persist.tile([P, MT, D], bf16)
            tt_sb = persist.tile([P, MT, D], bf16)
            zT_own = [persist.tile([P, K, S], fp8, name=f"zT_own{a}") for a in range(2)]
            stats = persist.tile([P, NSTAT], f32)
            colp_sb = stats[:, : 3 * G]
            nsq_own = stats[:, 3 * G : 3 * G + 3 * MT]
            dots_own = stats[:, 3 * G + 3 * MT : 3 * G + 6 * MT]
            poi = stats[:, 3 * G + 6 * MT : 3 * G + 8 * MT]
            scale16 = persist.tile([P, 2 * MT], f32)   # 16/||a|| for f1,f2 own
            bnsq = persist.tile([P, 2 * G], f32)       # ||b_n||^2 (fp8 data)
            bscale = persist.tile([P, 2 * G], f32)     # 1/(16*T*||b_n||)
            rows_acc = persist.tile([1, 3 * S], f32)

            # ---- DMA dispatch: fT chunks on the scalar HWDGE queue, own
            # features on sync; inp/tgt queue last on scalar (tail-only data).
            for b in (1, 0):
                for ch in range(NCH):
                    nc.scalar.dma_start(
                        fT_sb[b][ch],
                        fT_dram[b][ts(ch, P), :].rearrange("p (k n) -> p k n", k=K),
                    )
            for fi in range(3):
                nc.sync.dma_start(
                    own_sb[fi], own_dram[fi].rearrange("p (t d) -> p t d", t=MT)
                )
            nc.scalar.dma_start(it_sb, inp.rearrange("p (t d) -> p t d", t=MT))
            nc.scalar.dma_start(tt_sb, tgt.rearrange("p (t d) -> p t d", t=MT))

            nc.vector.memset(rows_acc, 0.0)

            # ---- phase A: own norms (f1/f2 only - f3's in the tail), ----
            for fi in range(2):
                for t in range(MT):
                    jt = junkp.tile([P, D], bf16, tag="junk16")
                    nc.vector.scalar_tensor_tensor(
                        out=jt, in0=own_sb[fi][:, t, :], scalar=1.0,
                        in1=own_sb[fi][:, t, :],
                        op0=ALU.mult, op1=ALU.mult,
                        accum_out=nsq_own[:, fi * MT + t : fi * MT + t + 1],
                    )

            # scale16 = 16 / ||a||  (ACT: exp(-0.5*ln(nsq) + ln 16))
            lnq = smallp.tile([P, 2 * MT], f32, tag="ln_own")
            nc.scalar.activation(lnq, nsq_own[:, : 2 * MT], AF.Ln)
            nc.scalar.activation(
                scale16, lnq, AF.Exp, scale=-0.5, bias=ln16_bias[:, :]
            )

            if cfg.use_ag:
                # share f2/f3 own-row norms across cores: 4KB AllGather.
                # bnsq[lane, b*G + c*MT + t] = ag_out[c*128+lane, b*MT+t]
                nc.sync.dma_start(ag_in, nsq_own[:, MT : 3 * MT])
                nc.gpsimd.collective_compute(
                    "AllGather",
                    mybir.AluOpType.bypass,
                    [list(range(cfg.n_cores))],
                    ins=[ag_in],
                    outs=[ag_out],
                )
                nc.sync.dma_start(
                    bnsq[:].rearrange("p (b c t) -> p b c t", b=2, c=cfg.n_cores),
                    ag_out.rearrange("(c p) (b t) -> p b c t", p=P, b=2),
                )
                lnb = smallp.tile([P, 2 * G], f32, tag="lnb_all")
                nc.scalar.activation(lnb, bnsq, AF.Ln)
                nc.scalar.activation(
                    bscale, lnb, AF.Exp, scale=-0.5, bias=lnbt_bias[:, :]
                )

            # fused normalize+transpose: one PE matmul per k-slice against
            # diag(16/||row||) (rows of identity scaled per-partition), so the
            # DVE never touches the D-wide data and the transposed result is
            # already normalized.  PSUM f32 -> fp8 in the copy out.
            KH = min(K, 4)  # k-slices per PSUM half-tile
            for fi in range(2):
                for t in range(MT):
                    diag = stage16.tile([P, P], bf16, tag="diag")
                    nc.vector.tensor_scalar_mul(
                        diag, ident16, scale16[:, fi * MT + t : fi * MT + t + 1]
                    )
                    for k0 in range(0, K, KH):
                        kw = min(KH, K - k0)
                        tps = ps_t.tile([P, KH * P], f32, tag="tps")
                        for k in range(kw):
                            nc.tensor.matmul(
                                tps[:, ts(k, P)],
                                own_sb[fi][:, t, ts(k0 + k, P)],
                                diag,
                                start=True, stop=True,
                            )
                        nc.vector.tensor_copy(
                            out=zT_own[fi][:, k0 : k0 + kw, ts(t, P)],
                            in_=tps[:, : kw * P].rearrange("p (k c) -> p k c", k=kw),
                        )

            def tail_tile(t):
                # f3 norms + diag dots + poisson partials for row-tile t
                jt0 = junkp.tile([P, D], bf16, tag="junk16")
                nc.vector.scalar_tensor_tensor(
                    out=jt0, in0=own_sb[2][:, t, :], scalar=1.0,
                    in1=own_sb[2][:, t, :],
                    op0=ALU.mult, op1=ALU.mult,
                    accum_out=nsq_own[:, 2 * MT + t : 2 * MT + t + 1],
                )
                for pi, (ia, ib) in enumerate(((0, 1), (0, 2), (1, 2))):
                    jt = junkp.tile([P, D], bf16, tag="junk16")
                    nc.vector.scalar_tensor_tensor(
                        out=jt, in0=own_sb[ia][:, t, :], scalar=1.0,
                        in1=own_sb[ib][:, t, :],
                        op0=ALU.mult, op1=ALU.mult,
                        accum_out=dots_own[:, pi * MT + t : pi * MT + t + 1],
                    )
                lg = lgp.tile([P, D], f32, tag="lg")
                nc.scalar.activation(lg, it_sb[:, t, :], AF.Ln, bias=eps_bias[:, :])
                jt = junkp.tile([P, D], bf16, tag="junk16")
                nc.vector.scalar_tensor_tensor(
                    out=jt, in0=tt_sb[:, t, :], scalar=1.0, in1=lg,
                    op0=ALU.mult, op1=ALU.mult,
                    accum_out=poi[:, MT + t : MT + t + 1],
                )
                jt2 = junkp.tile([P, D], bf16, tag="junk16")
                nc.vector.tensor_scalar(
                    out=jt2, in0=it_sb[:, t, :], scalar1=1.0, scalar2=0.0,
                    op0=ALU.mult, op1=ALU.add, accum_out=poi[:, t : t + 1],
                )

            # distribute the MT tail tiles over the b=1 chunks (ACT/DVE slack)
            tail_sched = {}
            for t in range(MT):
                tail_sched.setdefault(t * NCH // MT if NCH >= MT else t % NCH, []).append(t)

            # ---- phase B: stream both b features (f3 first: its 2 pairs
            # give the scheduler more work early; f2's single pair drains
            # faster at the end) ----
            for b in (1, 0):
                for ch in range(NCH):
                    if not cfg.use_ag:
                        # b-norms for this chunk: fp8 Gram diagonals
                        for s in range(SC):
                            g = ch * SC + s
                            gram = ps_g.tile([P, P], f32, tag="gram")
                            bsub = fT_sb[b][ch][:, :, ts(s, P)]
                            for j in range(KP):
                                nc.tensor.matmul(
                                    gram,
                                    bsub[:, 2 * j : 2 * j + 2, :],
                                    bsub[:, 2 * j : 2 * j + 2, :],
                                    start=(j == 0), stop=(j == KP - 1),
                                    perf_mode=DR,
                                )
                            j8 = junkp.tile([P, P], bf16, tag="junkg")
                            nc.vector.scalar_tensor_tensor(
                                out=j8, in0=gram, scalar=1.0, in1=ident32,
                                op0=ALU.mult, op1=ALU.mult,
                                accum_out=bnsq[:, b * G + g : b * G + g + 1],
                            )
                        lnb = smallp.tile([P, SC], f32, tag="lnb")
                        sl = slice(b * G + ch * SC, b * G + (ch + 1) * SC)
                        nc.scalar.activation(lnb, bnsq[:, sl], AF.Ln)
                        nc.scalar.activation(
                            bscale[:, sl], lnb, AF.Exp,
                            scale=-0.5, bias=lnbt_bias[:, :],
                        )

                    # sim matmuls + exp + row/col sums
                    for (pair, a) in pairs_of_b[b]:
                        e2 = None
                        rp = ps_rowp.tile([1, S], f32, tag="rp")
                        for s in range(SC):
                            g = ch * SC + s
                            ps = ps_s.tile([P, S], f32, tag="ps_s")
                            bsub = fT_sb[b][ch][:, :, ts(s, P)]
                            for j in range(KP):
                                nc.tensor.matmul(
                                    ps,
                                    bsub[:, 2 * j : 2 * j + 2, :],
                                    zT_own[a][:, 2 * j : 2 * j + 2, :],
                                    start=(j == 0), stop=(j == KP - 1),
                                    perf_mode=DR,
                                )
                            if s % 2 == 0:
                                e2 = expp.tile([P, 2, S], fp8, tag="exp8")
                            nc.scalar.activation(
                                e2[:, s % 2, :], ps, AF.Exp,
                                scale=bscale[:, b * G + g : b * G + g + 1],
                                accum_out=colp_sb[:, pair * G + g : pair * G + g + 1],
                            )
                            if s % 2 == 1:
                                nc.tensor.matmul(
                                    rp,
                                    ones8,
                                    e2[:, :, :],
                                    start=(s == 1),
                                    stop=(s == SC - 1),
                                    perf_mode=DR,
                                    skip_group_check=True,
                                )
                        acc = rows_acc[:, pair * S : (pair + 1) * S]
                        nc.vector.tensor_tensor(
                            out=acc, in0=rp, in1=acc, op=ALU.add
                        )
                    if b == 0:
                        for t in tail_sched.get(ch, []):
                            tail_tile(t)

            # ---- outputs (HWDGE, 2 DMAs) ----
            nc.sync.dma_start(rows_d, rows_acc)
            nc.sync.dma_start(stats_d, stats)

    nc.compile()
    return nc


def _tile_rows(a, MT):
    """[S, D] -> [P, MT*D] contiguous: row (t*128+p) -> out[p, t*D:(t+1)*D]."""
    S, D = a.shape
    return np.ascontiguousarray(
        a.reshape(MT, P, D).transpose(1, 0, 2).reshape(P, MT * D)
    )


def make_in_maps(cfg: Cfg, inputs, targets, feature1, feature2, feature3):
    import ml_dtypes

    bf16 = ml_dtypes.bfloat16
    fp8 = ml_dtypes.float8_e4m3
    K, NCH, NTC, MT = cfg.K, cfg.NCH, cfg.NTC, cfg.MT

    def prep_fT(f):
        # [B, D] f32 -> fp8 b^T pre-tiled [NCH*P, K*NTC]:
        # row (ch*P + p), col (k*NTC + n) = f[ch*NTC + n, k*P + p]
        t8 = np.ascontiguousarray(np.asarray(f, dtype=np.float32).T).astype(fp8)
        # t8: [D, B] = [K*P, NCH*NTC]
        return np.ascontiguousarray(
            t8.reshape(K, P, NCH, NTC).transpose(2, 1, 0, 3).reshape(NCH * P, K * NTC)
        )

    f2T8 = prep_fT(feature2)
    f3T8 = prep_fT(feature3)
    maps = []
    for c in range(cfg.n_cores):
        sl = slice(c * cfg.S, (c + 1) * cfg.S)
        maps.append({
            "f1_own": _tile_rows(np.asarray(feature1[sl]).astype(bf16), MT),
            "f2_own": _tile_rows(np.asarray(feature2[sl]).astype(bf16), MT),
            "f3_own": _tile_rows(np.asarray(feature3[sl]).astype(bf16), MT),
            "inp_own": _tile_rows(np.asarray(inputs[sl]).astype(bf16), MT),
            "tgt_own": _tile_rows(np.asarray(targets[sl]).astype(bf16), MT),
            "f2T8": f2T8,
            "f3T8": f3T8,
        })
    return maps


def combine_results(cfg: Cfg, per_core):
    """per_core: list of dicts with rowsums/colparts/nsq_own/dots_own/poi."""
    B, MT, S, G = cfg.B, cfg.MT, cfg.S, cfg.G
    nsq = np.zeros((3, B), np.float64)
    dots = np.zeros((3, B), np.float64)
    rowsum = np.zeros((3, B), np.float64)
    colsum = np.zeros((3, B), np.float64)
    poi_in = 0.0
    poi_tl = 0.0
    for c, r in enumerate(per_core):
        rs = np.asarray(r["rowsums"], np.float64).reshape(3, S)
        st = np.asarray(r["stats"], np.float64)
        cp = st[:, : 3 * G]
        nq = st[:, 3 * G : 3 * G + 3 * MT]
        dt_ = st[:, 3 * G + 3 * MT : 3 * G + 6 * MT]
        po = st[:, 3 * G + 6 * MT : 3 * G + 8 * MT]
        for fi in range(3):
            for t in range(MT):
                rows = slice(c * S + t * P, c * S + (t + 1) * P)
                nsq[fi, rows] = nq[:, fi * MT + t]
        for pi in range(3):
            rowsum[pi, c * S : (c + 1) * S] = rs[pi]
            for t in range(MT):
                rows = slice(c * S + t * P, c * S + (t + 1) * P)
                dots[pi, rows] = dt_[:, pi * MT + t]
            # colparts: n = g*128 + lane
            colsum[pi] += cp[:, pi * G : (pi + 1) * G].T.reshape(-1)
        poi_in += po[:, :MT].sum()
        poi_tl += po[:, MT:].sum()

    na = np.sqrt(nsq)  # [3, B]
    pairs = ((0, 1), (0, 2), (1, 2))
    closs = 0.0
    for pi, (ia, ib) in enumerate(pairs):
        simdiag = dots[pi] / (na[ia] * na[ib])
        loss_i = np.mean(np.log(rowsum[pi]) - simdiag / TEMPERATURE)
        loss_j = np.mean(np.log(colsum[pi]) - simdiag / TEMPERATURE)
        closs += 0.5 * (loss_i + loss_j)
    closs /= 3.0
    p_loss = (poi_in - poi_tl) / (cfg.B * cfg.D)
    total = p_loss + closs
    return (
        np.float32(total),
        np.float32(p_loss),
        np.float32(closs),
    )


_CACHE = {}


def _get_compiled(cfg: Cfg):
    key = (cfg.B, cfg.D, cfg.n_cores, cfg.NTC)
    if key not in _CACHE:
        _CACHE[key] = build_bass(cfg)
    return _CACHE[key]


def kernel(inputs, targets, feature1, feature2, feature3):
    from concourse.bass_utils import run_bass_kernel_spmd

    cfg = Cfg(B=inputs.shape[0], D=inputs.shape[1], n_cores=8)
    nc = _get_compiled(cfg)
    in_maps = make_in_maps(cfg, inputs, targets, feature1, feature2, feature3)
    res = run_bass_kernel_spmd(nc, in_maps, core_ids=list(range(cfg.n_cores)))
    return combine_results(cfg, res.results)


if __name__ == "__main__":
    # smoke test on hardware with full shapes
    rng = np.random.default_rng(0)
    B, D = 4096, 1024
    ins = {
        "inputs": rng.random((B, D), np.float32),
        "targets": rng.random((B, D), np.float32),
        "feature1": rng.standard_normal((B, D), np.float32),
        "feature2": rng.standard_normal((B, D), np.float32),
        "feature3": rng.standard_normal((B, D), np.float32),
    }
    out = kernel(**ins)
    print(out)
